# revision 5
# baseline (speedup 1.0000x reference)
"""DetectionLoss Bass kernel for TRN2, 8-core SPMD (vocab-sharded).

Device (identical program on all 8 cores; only the vocab slice of
caption_logits differs):
  * build the (64,256) fused POSITIVE cost matrix ncf = 16 - cost (both
    samples stacked on the partition dim) from host-prepped box rows,
  * 32-step greedy matching entirely on the vector engine:
      - per-row top-1 via max/max_index over (64,256); the index bits are
        written straight into the transpose staging tile (bitcast),
      - the used-gt mask is a per-partition accumulator folded into the
        staging column BEFORE the (64,64) stream transpose,
      - global per-sample argmax in the free dim; used-pred masking
        MULTIPLIES the cost column to zero via an iota compare
        (positive costs => zero never wins), no registers anywhere,
  * per step ONE indirect SWDGE gather (gpsimd ring) fetches the two
    matched predictions' caption slabs as 8 descriptors x 64KB (4
    caption rows per SBUF partition); the 8 row-offsets are produced by
    a one-hot PE matmul from the stored match + a host constant,
  * exp + per-subrange accumulate on ACT every 16 steps (4 sweeps of
    (128,4000), one per caption-row-within-partition) -> partial
    sum(exp) per (b,step,l) over this core's vocab slice; the unused
    l=15 row rides along and is ignored by the host.
Host: preps the broadcast box rows, shards caption_logits by vocab,
all-reduces the per-core partial sumexps, takes log, gathers target
token logits, and computes the scalar bbox/objectness losses and the
final weighted combination from the device-produced matching (pis,
gjs) - these are O(B*N) scalar reductions.
"""

import sys

sys.path.insert(0, "/opt/trn_rl_repo")

import numpy as np

import concourse.bacc as bacc
import concourse.mybir as mybir
from concourse.bass import IndirectOffsetOnAxis
from concourse.tile import TileContext

F32 = mybir.dt.float32
I32 = mybir.dt.int32
U32 = mybir.dt.uint32
Alu = mybir.AluOpType
Act = mybir.ActivationFunctionType
Eng = mybir.EngineType

B, N, M, L = 2, 256, 32, 16
LM1 = L - 1  # 15 caption positions
S = M  # greedy steps
NEGBIG = -1.0e9
EPS = 1e-7
RPG = 4  # caption rows per gathered partition
PPS = B * L // RPG  # 8 partitions (descriptors) per step
GSTEPS = 16  # steps per exp-sweep group
NGRP = S // GSTEPS  # 2 groups
NSEG = 10  # pbig segments


def build_nc(V8: int, num_devices: int = 8):
    """Build the per-core Bass program. V8 = vocab slice width per core."""
    nc = bacc.Bacc(
        "TRN2", target_bir_lowering=False, debug=False, num_devices=num_devices
    )

    cl = nc.dram_tensor("cl", (B * N * L, V8), F32, kind="ExternalInput")
    # pbig: per (b,j) partition, NSEG x 256 row segments:
    # [x1n y1n x2n y2n x1 y1 x2 y2 sig14 a1]   (sig14 = sigmoid(po)+14)
    pbig = nc.dram_tensor("pbig", (64, NSEG * N), F32, kind="ExternalInput")
    # gbx: per (b,j) partition: [gx1n gy1n gx2n gy2n ga2 g0 g1 g2 g3]
    gbx = nc.dram_tensor("gbx", (64, 9), F32, kind="ExternalInput")
    # wsel: one-hot weights for the offset matmul (64 x PPS)
    wsel = nc.dram_tensor("wsel", (64, PPS), F32, kind="ExternalInput")
    # cst: col0 = partition index % 32; col1 = lc (row offset consts) padded
    cst = nc.dram_tensor("cst", (64, 2), F32, kind="ExternalInput")

    outse = nc.dram_tensor("outse", (128, RPG * NGRP), F32, kind="ExternalOutput")
    pis_o = nc.dram_tensor("pis_o", (64, S), F32, kind="ExternalOutput")
    gjs_o = nc.dram_tensor("gjs_o", (64, 8 * S), U32, kind="ExternalOutput")

    # DRAM view for the indirect gather: row = 4 consecutive caption rows
    cl4 = cl[:].rearrange("(r x) v -> r (x v)", x=RPG)  # (2048, 4*V8)

    with TileContext(nc) as tc:
        with (
            tc.tile_pool(name="cpool", bufs=1) as cp,
            tc.tile_pool(name="gpool", bufs=2) as gp,
            tc.tile_pool(name="dpool", bufs=1) as dp,
            tc.tile_pool(name="ppool", bufs=2, space="PSUM") as pp,
            tc.tile_pool(name="opool", bufs=2) as op,
        ):
            ts = nc.vector.tensor_scalar
            tt = nc.vector.tensor_tensor
            stt = nc.vector.scalar_tensor_tensor

            # ---------- input loads ----------
            pbig_sb = cp.tile([64, NSEG * N], F32)
            nc.sync.dma_start(pbig_sb[:], pbig[:])
            gbx_sb = cp.tile([64, 9], F32)
            nc.sync.dma_start(gbx_sb[:], gbx[:])
            wsel_sb = cp.tile([64, PPS], F32)
            nc.sync.dma_start(wsel_sb[:], wsel[:])
            cst_sb = cp.tile([64, 2], F32)
            nc.sync.dma_start(cst_sb[:], cst[:])

            def seg(k):
                return pbig_sb[:, k * N : (k + 1) * N]

            def gcol(k):
                return gbx_sb[:, k : k + 1]

            # iota row constant 0..255 on every partition
            iota0 = cp.tile([64, N], F32)
            nc.gpsimd.iota(
                iota0[:], pattern=[[1, N]], base=0, channel_multiplier=0,
                allow_small_or_imprecise_dtypes=True,
            )

            # ---------- cost matrix: ncf = 16 - cost  (all positive) ----------
            xi2 = cp.tile([64, N], F32)
            yi2 = cp.tile([64, N], F32)
            iw = cp.tile([64, N], F32)
            ih = cp.tile([64, N], F32)
            inter = cp.tile([64, N], F32)
            ts(xi2[:], seg(2), gcol(2), None, op0=Alu.min)
            stt(iw[:], seg(0), gcol(0), xi2[:], op0=Alu.max, op1=Alu.subtract)
            ts(iw[:], iw[:], -1.0, 0.0, op0=Alu.mult, op1=Alu.max)
            ts(yi2[:], seg(3), gcol(3), None, op0=Alu.min)
            stt(ih[:], seg(1), gcol(1), yi2[:], op0=Alu.max, op1=Alu.subtract)
            ts(ih[:], ih[:], -1.0, 0.0, op0=Alu.mult, op1=Alu.max)
            tt(inter[:], iw[:], ih[:], op=Alu.mult)

            ew = cp.tile([64, N], F32)
            eh = cp.tile([64, N], F32)
            enc = cp.tile([64, N], F32)
            ts(ew[:], seg(2), gcol(2), None, op0=Alu.max)
            stt(ew[:], seg(0), gcol(0), ew[:], op0=Alu.min, op1=Alu.subtract)
            ts(eh[:], seg(3), gcol(3), None, op0=Alu.max)
            stt(eh[:], seg(1), gcol(1), eh[:], op0=Alu.min, op1=Alu.subtract)
            tt(enc[:], ew[:], eh[:], op=Alu.mult)  # (-ew)*(-eh) = enc

            union = cp.tile([64, N], F32)
            stt(union[:], seg(9), gcol(4), inter[:], op0=Alu.add,
                op1=Alu.subtract)

            r1 = cp.tile([64, N], F32)
            r2 = cp.tile([64, N], F32)
            ts(r1[:], union[:], EPS, None, op0=Alu.add)
            nc.vector.reciprocal(r1[:], r1[:])
            ts(r2[:], enc[:], EPS, None, op0=Alu.add)
            nc.vector.reciprocal(r2[:], r2[:])

            giou = cp.tile([64, N], F32)
            tt(giou[:], inter[:], r1[:], op=Alu.mult)  # iou
            tt(enc[:], enc[:], union[:], op=Alu.subtract)  # enc - union
            tt(enc[:], enc[:], r2[:], op=Alu.mult)
            tt(giou[:], giou[:], enc[:], op=Alu.subtract)

            # l1 via strided diff tile + abs-reduce
            ld = cp.tile([64, 4 * N], F32)
            ldv = ld[:].rearrange("p (i c) -> p i c", c=4)
            for c in range(4):
                ts(ldv[:, :, c], seg(4 + c), gcol(5 + c), None,
                   op0=Alu.subtract)
            l1s = cp.tile([64, N], F32)
            nc.vector.tensor_reduce(
                l1s[:], ldv[:, :, :], axis=mybir.AxisListType.X, op=Alu.add,
                apply_absolute_value=True,
            )

            ncf = cp.tile([64, N], F32)
            stt(ncf[:], l1s[:], -1.0, giou[:], op0=Alu.mult, op1=Alu.add)
            tt(ncf[:], ncf[:], seg(8), op=Alu.add)

            # ---------- greedy matching ----------
            tile64 = cp.tile([64, 64], F32)
            nc.vector.memset(tile64[:], 0.0)
            T = cp.tile([64, 64], F32)
            pk8 = cp.tile([64, 8], F32)
            gtmT = cp.tile([64, 1], F32)
            nc.vector.memset(gtmT[:], 0.0)
            g8 = cp.tile([64, 8], F32)
            tsel = cp.tile([64, 32], F32)
            i1 = cp.tile([64, 8], F32)
            i1b = cp.tile([64, 1], F32)
            jf = cp.tile([64, 1], F32)
            j1b = cp.tile([64, 1], F32)
            ohj = cp.tile([64, 1], F32)
            gjsall = cp.tile([64, 8 * S], U32)
            pisrf = cp.tile([64, S], F32)
            outse_sb = cp.tile([128, RPG * NGRP], F32)

            for s in range(S):
                w, k = divmod(s, GSTEPS)
                # --- critical chain ---
                nc.vector.max(pk8[:], ncf[:])
                ts(tile64[:, 0:1], pk8[:, 0:1], gtmT[:, 0:1], None, op0=Alu.add)
                nc.vector.max_index(
                    tile64[:, 32:40].bitcast(U32), pk8[:], ncf[:])
                nc.vector.transpose(T[:], tile64[:])
                nc.vector.max(g8[:], T[:, 0:32])
                stt(tsel[:], T[:, 0:32], g8[:, 0:1], T[:, 32:64].bitcast(U32),
                    op0=Alu.is_equal, op1=Alu.mult)
                nc.vector.max(i1[:], tsel[:])
                nc.vector.stream_shuffle(i1b[:], i1[:, 0:1], [0] * 32)
                stt(ncf[:], iota0[:], i1b[:, 0:1], ncf[:],
                    op0=Alu.not_equal, op1=Alu.mult)
                # --- off-chain: gt mask + outputs ---
                nc.vector.max_index(gjsall[:, 8 * s : 8 * s + 8], g8[:], T[:, 0:32])
                ts(jf[:], gjsall[:, 8 * s : 8 * s + 1], 0.0, None, op0=Alu.add)
                nc.vector.stream_shuffle(j1b[:], jf[:], [0] * 32)
                ts(ohj[:], cst_sb[:, 0:1], j1b[:, 0:1], None, op0=Alu.is_equal)
                stt(gtmT[:], ohj[:], NEGBIG, gtmT[:], op0=Alu.mult, op1=Alu.add)
                nc.vector.tensor_copy(pisrf[:, s : s + 1], i1[:, 0:1])

                # --- offsets via PE one-hot + indirect SWDGE gather ---
                psum8 = pp.tile([PPS, 1], F32, tag="psum8")
                nc.tensor.matmul(
                    psum8[:], wsel_sb[:], pisrf[:, s : s + 1],
                    start=True, stop=True)
                offi = op.tile([PPS, 1], I32, tag="offi")
                tt(offi[:], psum8[:], cst_sb[0:PPS, 1:2], op=Alu.add)
                if k == 0:
                    gtile = gp.tile([128, RPG * V8], F32, tag="gtile")
                nc.gpsimd.indirect_dma_start(
                    out=gtile[PPS * k : PPS * (k + 1), :],
                    out_offset=None,
                    in_=cl4,
                    in_offset=IndirectOffsetOnAxis(ap=offi[:, 0:1], axis=0),
                )
                if k == GSTEPS - 1:
                    dump = dp.tile([128, V8], F32, tag="dump")
                    for c in range(RPG):
                        nc.scalar.activation(
                            dump[:], gtile[:, c * V8 : (c + 1) * V8], Act.Exp,
                            accum_out=outse_sb[:, RPG * w + c : RPG * w + c + 1])

            # ---------- outputs ----------
            nc.sync.dma_start(outse[:], outse_sb[:])
            nc.sync.dma_start(pis_o[:], pisrf[:])
            nc.sync.dma_start(gjs_o[:], gjsall[:])

    nc.compile()
    return nc


# ---------------- host side ----------------

def shard_inputs(pred_boxes, pred_objectness, caption_logits, gt_boxes, V8, NC=8):
    pbf = pred_boxes.astype(np.float32)
    po = pred_objectness.astype(np.float32)
    x1n = np.minimum(pbf[..., 0], pbf[..., 2])
    y1n = np.minimum(pbf[..., 1], pbf[..., 3])
    x2n = np.maximum(pbf[..., 0], pbf[..., 2])
    y2n = np.maximum(pbf[..., 1], pbf[..., 3])
    sig14 = (1.0 / (1.0 + np.exp(-po)) + 14.0).astype(np.float32)
    a1 = ((x2n - x1n) * (y2n - y1n)).astype(np.float32)
    rows = np.stack(
        [x1n, y1n, x2n, y2n, pbf[..., 0], pbf[..., 1], pbf[..., 2],
         pbf[..., 3], sig14, a1], axis=1)  # (B, NSEG, N)
    pbig = np.broadcast_to(rows[:, None, :, :], (B, M, NSEG, N)).reshape(
        64, NSEG * N)
    pbig = np.ascontiguousarray(pbig)

    gbf = gt_boxes.astype(np.float32)
    gx1n = np.minimum(gbf[..., 0], gbf[..., 2])
    gy1n = np.minimum(gbf[..., 1], gbf[..., 3])
    gx2n = np.maximum(gbf[..., 0], gbf[..., 2])
    gy2n = np.maximum(gbf[..., 1], gbf[..., 3])
    ga2 = (gx2n - gx1n) * (gy2n - gy1n)
    gbx = np.stack(
        [gx1n, gy1n, gx2n, gy2n, ga2, gbf[..., 0], gbf[..., 1], gbf[..., 2],
         gbf[..., 3]], axis=-1).reshape(64, 9).astype(np.float32)
    gbx = np.ascontiguousarray(gbx)

    # offset matmul weights: psum[p<4] = 4*i_A, psum[p>=4] = 4*i_B
    wsel = np.zeros((64, PPS), np.float32)
    wsel[0, 0:PPS // 2] = float(RPG)
    wsel[32, PPS // 2 :] = float(RPG)
    cst = np.zeros((64, 2), np.float32)
    cst[:, 0] = np.arange(64) % 32
    lc = np.concatenate(
        [np.arange(PPS // 2), N * L // RPG + np.arange(PPS // 2)])
    cst[0:PPS, 1] = lc.astype(np.float32)

    clv = caption_logits.reshape(B * N * L, NC, V8)
    in_maps = []
    for c in range(NC):
        in_maps.append({
            "cl": np.ascontiguousarray(clv[:, c, :]).astype(np.float32, copy=False),
            "pbig": pbig, "gbx": gbx, "wsel": wsel, "cst": cst,
        })
    return in_maps


def _giou_np(b1, b2):
    def norm(b):
        x1 = np.minimum(b[..., 0], b[..., 2]); y1 = np.minimum(b[..., 1], b[..., 3])
        x2 = np.maximum(b[..., 0], b[..., 2]); y2 = np.maximum(b[..., 1], b[..., 3])
        return x1, y1, x2, y2
    ax1, ay1, ax2, ay2 = norm(b1)
    bx1, by1, bx2, by2 = norm(b2)
    xi1 = np.maximum(ax1, bx1); yi1 = np.maximum(ay1, by1)
    xi2 = np.minimum(ax2, bx2); yi2 = np.minimum(ay2, by2)
    inter = np.clip(xi2 - xi1, 0.0, None) * np.clip(yi2 - yi1, 0.0, None)
    a1 = (ax2 - ax1) * (ay2 - ay1)
    a2 = (bx2 - bx1) * (by2 - by1)
    union = a1 + a2 - inter
    iou = inter / (union + EPS)
    xe1 = np.minimum(ax1, bx1); ye1 = np.minimum(ay1, by1)
    xe2 = np.maximum(ax2, bx2); ye2 = np.maximum(ay2, by2)
    enc = (xe2 - xe1) * (ye2 - ye1)
    return iou - (enc - union) / (enc + EPS)


def combine(results, caption_logits, gt_tokens, pred_boxes, pred_objectness,
            gt_boxes, V8, NC=8):
    """results: list of per-core dicts with outse/pis_o/gjs_o."""
    out0 = results[0]
    sums = np.zeros((128, RPG * NGRP), np.float64)
    for c in range(NC):
        sums += results[c]["outse"].astype(np.float64)
    # partition p = 8k + 4b + r, col = RPG*w + c -> (b, s=16w+k, l=4r+c)
    lse_bsl = np.zeros((B, S, LM1))
    pidx = np.arange(128)
    kk, bb, rr = pidx // 8, (pidx % 8) // 4, pidx % 4
    for w in range(NGRP):
        for c in range(RPG):
            l = 4 * rr + c
            valid = l < LM1
            lse_bsl[bb[valid], 16 * w + kk[valid], l[valid]] = np.log(
                sums[pidx[valid], RPG * w + c])

    pis = np.rint(out0["pis_o"][[0, 32], :]).astype(np.int64)  # (2, 32)
    gjs = out0["gjs_o"][[0, 32], ::8].astype(np.int64)  # (2, 32)
    tok = np.asarray(gt_tokens).astype(np.int64)

    bidx = np.arange(B)[:, None, None]
    lidx = np.arange(LM1)[None, None, :]
    tgt = tok[bidx, gjs[:, :, None], lidx + 1]  # (B, S, LM1)
    tlog = caption_logits[bidx, pis[:, :, None], lidx, tgt].astype(np.float64)
    ce = (lse_bsl - tlog).mean(axis=2)  # (B, S)
    cap = np.clip(np.clip(ce, 0.0, None).mean(axis=1), 0.0, None)  # (B,)

    pb = np.asarray(pred_boxes, np.float64)
    gb = np.asarray(gt_boxes, np.float64)
    po = np.asarray(pred_objectness, np.float64)
    bbox = np.zeros(B); obj = np.zeros(B)
    for b in range(B):
        mp = pb[b][pis[b]]; mg = gb[b][gjs[b]]
        l1_loss = np.abs(mp - mg).mean()
        giou_loss = np.clip((1.0 - _giou_np(mp, mg)).mean(), 0.0, 2.0)
        bbox[b] = max(l1_loss + giou_loss, 0.0)
        t = np.zeros(N); t[pis[b]] = 1.0
        ob = (np.maximum(po[b], 0.0) - po[b] * t
              + np.log1p(np.exp(-np.abs(po[b])))).mean()
        obj[b] = max(ob, 0.0)

    total = max((5.0 * bbox + 0.1 * cap + obj).mean(), 0.0)
    comps = [5.0 * bbox.mean(), 0.1 * cap.mean(), obj.mean()]
    return np.array([total] + comps, np.float32)


# ---------------- entry points ----------------

V8_FULL = 4000
NC_CORES = 8
_CACHE = {}


def get_nc(V8=V8_FULL):
    key = V8
    if key not in _CACHE:
        _CACHE[key] = build_nc(V8, num_devices=NC_CORES)
    return _CACHE[key]


def run_device(in_maps, V8=V8_FULL, trace=False, **kw):
    from concourse.bass_utils import run_bass_kernel_spmd

    nc = get_nc(V8)
    return run_bass_kernel_spmd(
        nc, in_maps, core_ids=list(range(NC_CORES)), trace=trace, **kw)


def kernel(pred_boxes, pred_objectness, caption_logits, gt_boxes, gt_tokens):
    pred_boxes = np.asarray(pred_boxes, np.float32)
    pred_objectness = np.asarray(pred_objectness, np.float32)
    caption_logits = np.asarray(caption_logits, np.float32)
    gt_boxes = np.asarray(gt_boxes, np.float32)
    in_maps = shard_inputs(
        pred_boxes, pred_objectness, caption_logits, gt_boxes, V8_FULL, NC_CORES)
    res = run_device(in_maps)
    return combine(res.results, caption_logits, gt_tokens, pred_boxes,
                   pred_objectness, gt_boxes, V8_FULL, NC_CORES)
